# revision 9
# baseline (speedup 1.0000x reference)
"""Trainium2 Bass kernel for nn_MultiHeadAttention (B=4, S=2048, D=1024, H=16, causal).

Sharding: 8 cores = 4 batches x 2 head-halves (8 heads each). Every core runs an
identical SPMD program: Q/K/V projections for its 8 heads over its batch's 2048
tokens, causal flash-attention, and a partial output projection over its 512
head-dims. Host unshard adds the two partial outputs per batch (+ bo).

All matmul operands are bf16 (fp32 PSUM accumulation). The softmax denominator
is fused into the PV matmul via a ones-column appended to each head's V block
(V tiles are [128 tok, 8 heads, 96]: cols 0-63 = V, col 64 = 1.0, rest 0 —
matmul output partition counts must be multiples of 32). Denominators are
reciprocal'd in place (DVE ops cannot shift partitions downward), broadcast to
all partitions via K=1 matmuls, and applied with scalar_tensor_tensor.
Diagonal-crossing k-chunks narrow their score/exp/PV ops to the unmasked
q-range. Projections for q-tile t+1, pair tails, and the out-projection of
tile t-1 are interleaved into tile t's chunk stream through a priority work
queue so the PE stays fed while the ACT engine grinds exp (the per-chunk
critical resource). Weight/x loads are single rearranged DMAs ([1024,512] ->
[128,8,512]) to cut HWDGE descriptor cost; attention-phase constants load
behind the first projection tiles.
"""

import os
import sys

for _p in ("/opt/trn_rl_repo", "/root/.axon_site/_ro/trn_rl_repo"):
    if os.path.isdir(_p) and _p not in sys.path:
        sys.path.insert(0, _p)

import numpy as np

B, S, D, H = 4, 2048, 1024, 16
HD = D // H  # 64
DH = D // 2  # 512 dims per head-half
NCORES = 8
QT_TILES = 4      # 512-token q tiles
PAIRS = 4         # head pairs per core (8 heads)
ICHUNKS = 8       # 128-row feature chunks of D
TT16 = 16         # 128-token tiles


def _build_nc(repeat=1):
    import concourse.mybir as mybir
    import concourse.tile as tile
    from concourse import bacc

    F32 = mybir.dt.float32
    F32R = mybir.dt.float32r
    BF16 = mybir.dt.bfloat16
    ACTF = mybir.ActivationFunctionType
    ALU = mybir.AluOpType

    nc = bacc.Bacc("TRN2", target_bir_lowering=False, debug=False, num_devices=NCORES)

    xqT = nc.dram_tensor("xqT", [D, S], BF16, kind="ExternalInput")
    xkT = nc.dram_tensor("xkT", [D, S], BF16, kind="ExternalInput")
    xvT = nc.dram_tensor("xvT", [D, S], BF16, kind="ExternalInput")
    wqt = nc.dram_tensor("wqt", [D, DH], BF16, kind="ExternalInput")
    wkt = nc.dram_tensor("wkt", [D, DH], BF16, kind="ExternalInput")
    wvt = nc.dram_tensor("wvt", [D, DH], BF16, kind="ExternalInput")
    wot = nc.dram_tensor("wot", [DH, D], BF16, kind="ExternalInput")
    bq = nc.dram_tensor("bq", [DH], F32, kind="ExternalInput")
    bk = nc.dram_tensor("bk", [DH], F32, kind="ExternalInput")
    bv = nc.dram_tensor("bv", [1, DH], BF16, kind="ExternalInput")
    onescol = nc.dram_tensor("onescol", [1, 128], BF16, kind="ExternalInput")
    seld = nc.dram_tensor("selp", [65, 64], F32R, kind="ExternalInput")
    masksd = nc.dram_tensor("masks", [4, 128, 1024], BF16, kind="ExternalInput")
    outp = nc.dram_tensor("outp", [S, D], BF16, kind="ExternalOutput")

    with tile.TileContext(nc) as tc:
        with (
            tc.tile_pool(name="const", bufs=1) as cp,
            tc.tile_pool(name="persist", bufs=1) as pp,
        ):
            onescol_t = cp.tile([1, 128], BF16, tag="onescol", name="onescol_t")
            selp_t = cp.tile([65, 64], F32R, tag="selp", name="selp_t")
            bv_t = cp.tile([1, DH], BF16, tag="bv", name="bv_t")
            mask_t = []
            for m in range(4):
                mt = cp.tile([128, 1024], BF16, tag=f"mask{m}", name=f"mask_t{m}")
                mask_t.append(mt)
            bq_t, bk_t = [], []
            for p in range(PAIRS):
                bq_t.append(cp.tile([128, 1], F32, tag=f"bq{p}", name=f"bq_t{p}"))
                bk_t.append(cp.tile([128, 1], F32, tag=f"bk{p}", name=f"bk_t{p}"))

            def load_consts_early():
                nc.sync.dma_start(onescol_t[:], onescol.ap())
                nc.sync.dma_start(bv_t[:], bv.ap())
                for p in range(PAIRS):
                    nc.sync.dma_start(bq_t[p][:], bq.ap()[128 * p : 128 * (p + 1)])
                    nc.sync.dma_start(bk_t[p][:], bk.ap()[128 * p : 128 * (p + 1)])

            def load_consts_late():
                # attention-phase consts: behind all projection loads
                nc.sync.dma_start(selp_t[:], seld.ap())
                for m in range(4):
                    nc.sync.dma_start(mask_t[m][:], masksd.ap()[m])
                for p in range(PAIRS):
                    nc.sync.dma_start(wo_t[p][:], wot.ap()[128 * p : 128 * (p + 1), :])
            # wo resident: 4 chunk tiles [128 d, 1024 e] (DMA deferred past first proj loads)
            wo_t = []
            for p in range(PAIRS):
                wt = pp.tile([128, D], BF16, tag=f"wo{p}", name=f"wo_t{p}")
                wo_t.append(wt)

            # persistent activations (feature-major Q/K; token-major V)
            QT = [pp.tile([128, S], BF16, tag=f"qt{p}", name=f"QT{p}") for p in range(PAIRS)]
            KT = [pp.tile([128, S], BF16, tag=f"kt{p}", name=f"KT{p}") for p in range(PAIRS)]
            # V: [128 tok, 8 head-groups, 128]; col 64 of each group stays 1.0.
            # 128-wide weight slices turn on FWL (fast weight load) for PV matmuls.
            V = [pp.tile([128, 8, 128], BF16, tag=f"v{i}", name=f"V{i}") for i in range(TT16)]
            for i in range(TT16):
                nc.vector.memset(V[i][:, :, 64:128], 0.0)
                nc.vector.memset(V[i][:, :, 64:65], 1.0)

            for _rep in range(repeat):
              with (
                  tc.tile_pool(name="xtb", bufs=4) as xtbp,
                  tc.tile_pool(name="wp", bufs=3) as wp,
                  tc.tile_pool(name="pbp", bufs=6) as pbp,
                  tc.tile_pool(name="rpool", bufs=3) as rpool,
                  tc.tile_pool(name="apool", bufs=3) as apool,
                  tc.tile_pool(name="osb", bufs=4) as osbp,
                  tc.tile_pool(name="sps", bufs=2, space="PSUM") as sps,
                  tc.tile_pool(name="acc", bufs=2, space="PSUM") as accps,
                  tc.tile_pool(name="shp", bufs=2, space="PSUM") as shps,
              ):
                  urgent, projq, lateq, normal = [], [], [], []
                  pace = {"iter": 0, "total": 1, "next_norm": 0}

                  def drain_one():
                      # urgent (softmax tails), proj prerequisites, and late
                      # K/V groups drain immediately; out-proj units are spaced
                      # across the tile so late ACT-bound chunks keep PE fed
                      for q in (urgent, projq, lateq):
                          if q:
                              q.pop(0)()
                              return
                      if normal and pace["iter"] >= pace["next_norm"]:
                          step = max(1, pace["total"] // 10)
                          pace["next_norm"] = pace["iter"] + step
                          normal.pop(0)[1]()

                  def load_w(wdram, split=False):
                      # one DMA: dram [(c p), q] -> sbuf [p, c, q]
                      wt = wp.tile([128, ICHUNKS, DH], BF16, tag="w", name="w_t")
                      src = wdram.ap().rearrange("(c p) q -> p c q", c=ICHUNKS)
                      if split:
                          # chunk 0 lands first so the first proj matmul can start
                          nc.sync.dma_start(wt[:, 0:2, :], src[:, 0:2, :])
                          nc.sync.dma_start(wt[:, 2:ICHUNKS, :], src[:, 2:ICHUNKS, :])
                      else:
                          nc.sync.dma_start(wt[:], src)
                      return wt

                  def dma_x(xdram, t, split=False):
                      tsl = slice(512 * t, 512 * (t + 1))
                      xt = xtbp.tile([128, ICHUNKS, 512], BF16, tag="xtb", name="xtb_tile")
                      src = xdram.ap()[:, tsl].rearrange("(c p) q -> p c q", c=ICHUNKS)
                      if split:
                          nc.sync.dma_start(xt[:, 0:2, :], src[:, 0:2, :])
                          nc.sync.dma_start(xt[:, 2:ICHUNKS, :], src[:, 2:ICHUNKS, :])
                      else:
                          nc.sync.dma_start(xt[:], src)
                      return xt

                  def qk_group(w_t, xtb, dst, bias_tiles, t, p, on_act=False):
                      pg = shps.tile([128, 512], F32, tag="sh", name="pg_t")
                      for c in range(ICHUNKS):
                          nc.tensor.matmul(
                              pg[:], w_t[:, c, 128 * p : 128 * (p + 1)], xtb[:, c, :],
                              start=(c == 0), stop=(c == ICHUNKS - 1),
                          )
                      if on_act:
                          # PSUM->SBUF + per-partition bias on the ACT engine
                          # (identity shares the exp activation table: no reload)
                          nc.scalar.activation(
                              dst[p][:, 512 * t : 512 * (t + 1)], pg[:],
                              ACTF.Identity, bias=bias_tiles[p][:], scale=1.0)
                      else:
                          nc.vector.tensor_scalar_add(
                              dst[p][:, 512 * t : 512 * (t + 1)], pg[:], bias_tiles[p][:])

                  def v_group(wv_t, xtb, t, b):
                      pg = shps.tile([128, 512], F32, tag="sh", name="pg_t")
                      for c in range(ICHUNKS):
                          nc.tensor.matmul(
                              pg[:], xtb[:, c, 128 * b : 128 * (b + 1)], wv_t[:, c, :],
                              start=(c == 0), stop=(c == ICHUNKS - 1),
                          )
                      nc.vector.scalar_tensor_tensor(
                          V[4 * t + b][:, :, 0:64], pg[:], 1.0, bvb[:],
                          ALU.mult, ALU.add,
                      )

                  # ---- startup: K weights + K x-tile first, consts behind ----
                  wk_t = load_w(wkt, split=(_rep == 0))
                  xk = dma_x(xkT, 0, split=(_rep == 0))
                  if _rep == 0:
                      load_consts_early()
                  wq_t = load_w(wqt)
                  xq = dma_x(xqT, 0)
                  wv_t = load_w(wvt)
                  xv = dma_x(xvT, 0)
                  if _rep == 0:
                      load_consts_late()

                  # ---- tile-0 projections emitted directly ----
                  for p in range(PAIRS):
                      qk_group(wk_t, xk, KT, bk_t, 0, p, on_act=True)
                  for p in range(PAIRS):
                      qk_group(wq_t, xq, QT, bq_t, 0, p)
                  # bv broadcast tile [128, 512]
                  pbv = shps.tile([128, 512], F32, tag="sh", name="pbv_tile")
                  nc.tensor.matmul(pbv[:], onescol_t[:], bv_t[:], start=True, stop=True)
                  bvb = wp.tile([128, DH], BF16, tag="bvb", name="bvb_tile")
                  nc.vector.tensor_copy(bvb[:], pbv[:])
                  for b in range(4):
                      v_group(wv_t, xv, 0, b)

                  def queue_proj(t1):
                      st = {}
                      kv = []
                      def dq(): st["xq"] = dma_x(xqT, t1)
                      projq.append(dq)
                      for p in range(PAIRS):
                          projq.append(lambda p=p: qk_group(wq_t, st["xq"], QT, bq_t, t1, p))
                      def dk(): st["xk"] = dma_x(xkT, t1)
                      kv.append(dk)
                      for p in range(PAIRS):
                          kv.append(lambda p=p: qk_group(
                              wk_t, st["xk"], KT, bk_t, t1, p, on_act=True))
                      def dv(): st["xv"] = dma_x(xvT, t1)
                      kv.append(dv)
                      for b in range(4):
                          kv.append(lambda b=b: v_group(wv_t, st["xv"], t1, b))
                      if t1 == QT_TILES - 1:
                          # last tile: its own chunks only touch K/V tile t1 from
                          # chunk j=4*t1 (iter 13) on; safe to drain in-stream
                          lateq.extend(kv)
                      else:
                          projq.extend(kv)

                  # ---- attention with interleaved proj/out-proj ----
                  for t in range(QT_TILES):
                      while projq:       # proj for this tile must be emitted
                          projq.pop(0)()
                      while normal and normal[0][0] <= t - 2:
                          # out-proj units two tiles back must emit before this
                          # tile's pair tails reuse their `a` buffers
                          normal.pop(0)[1]()
                      if t < QT_TILES - 1:
                          queue_proj(t + 1)
                      nch = 4 * (t + 1)
                      pace["iter"], pace["total"], pace["next_norm"] = 0, 4 * nch, 0
                      A = []
                      for p in range(PAIRS):
                          psA = accps.tile([128, 512], F32, tag="acc", name="psA_t")
                          psB = accps.tile([128, 512], F32, tag="acc", name="psB_t")
                          for j in range(nch):
                              ksl = slice(128 * j, 128 * (j + 1))
                              m = j - 4 * t
                              q0 = 128 * m if m >= 1 else 0
                              s01 = sps.tile([128, 1024], F32, tag="s01", name="s01_t")
                              nc.tensor.matmul(
                                  s01[:, q0:512], KT[p][0:64, ksl],
                                  QT[p][0:64, 512 * t + q0 : 512 * (t + 1)],
                                  start=True, stop=True,
                              )
                              nc.tensor.matmul(
                                  s01[:, 512 + q0 : 1024], KT[p][64:128, ksl],
                                  QT[p][64:128, 512 * t + q0 : 512 * (t + 1)],
                                  start=True, stop=True,
                              )
                              pb = pbp.tile([128, 1024], BF16, tag="pb", name="pb_t")
                              if q0 == 0:
                                  nc.scalar.activation(pb[:], s01[:], ACTF.Exp, scale=0.125)
                              else:
                                  nc.scalar.activation(
                                      pb[:, q0:512], s01[:, q0:512], ACTF.Exp, scale=0.125)
                                  nc.scalar.activation(
                                      pb[:, 512 + q0 : 1024], s01[:, 512 + q0 : 1024],
                                      ACTF.Exp, scale=0.125)
                              if m >= 0:
                                  if q0 == 0:
                                      nc.vector.tensor_tensor(
                                          pb[:], pb[:], mask_t[m][:], ALU.mult)
                                  else:
                                      nc.vector.tensor_tensor(
                                          pb[:, q0:512], pb[:, q0:512],
                                          mask_t[m][:, q0:512], ALU.mult)
                                      nc.vector.tensor_tensor(
                                          pb[:, 512 + q0 : 1024], pb[:, 512 + q0 : 1024],
                                          mask_t[m][:, 512 + q0 : 1024], ALU.mult)
                              st_ = (j == 0)
                              sp_ = (j == nch - 1)
                              nc.tensor.matmul(
                                  psA[:, q0:512], V[j][:, 2 * p : 2 * p + 1, :],
                                  pb[:, q0:512], start=st_, stop=sp_,
                              )
                              nc.tensor.matmul(
                                  psB[:, q0:512], V[j][:, 2 * p + 1 : 2 * p + 2, :],
                                  pb[:, 512 + q0 : 1024], start=st_, stop=sp_,
                              )
                              pace["iter"] += 1
                              drain_one()

                          def pair_tail(p=p, psA=psA, psB=psB, A=A):
                              r2 = rpool.tile([65, 512], F32R, tag="r", name="r_t")
                              r2b = rpool.tile([65, 512], F32R, tag="r", name="r2b_t")
                              with nc.allow_low_precision(reason="f32r storage is fp32"):
                                  nc.vector.reciprocal(r2[64:65, :], psA[64:65, :])
                                  nc.vector.reciprocal(r2b[64:65, :], psB[64:65, :])
                              rbA = shps.tile([64, 512], F32, tag="sh", name="rbA_t")
                              rbB = shps.tile([64, 512], F32, tag="sh", name="rbB_t")
                              nc.tensor.matmul(rbA[:], selp_t[64:65, :], r2[64:65, :],
                                               start=True, stop=True)
                              nc.tensor.matmul(rbB[:], selp_t[64:65, :], r2b[64:65, :],
                                               start=True, stop=True)
                              rbc = rpool.tile([128, 512], BF16, tag="rbc", name="rbc_t")
                              nc.vector.tensor_copy(rbc[0:64, :], rbA[:])
                              nc.vector.tensor_scalar_mul(rbc[64:128, :], rbB[:], 1.0)
                              a = apool.tile([128, 512], BF16, tag=f"a{p}", name=f"a_t{p}")
                              nc.vector.scalar_tensor_tensor(
                                  a[0:64, :], psA[0:64, :], 1.0, rbc[0:64, :],
                                  ALU.mult, ALU.mult,
                              )
                              nc.vector.scalar_tensor_tensor(
                                  a[64:128, :], psB[0:64, :], 1.0, rbc[64:128, :],
                                  ALU.mult, ALU.mult,
                              )
                              A.append(a)

                          urgent.append(pair_tail)

                      for tl in range(4):
                          for eh in range(2):
                              def oproj(t=t, tl=tl, eh=eh, A=A):
                                  po = shps.tile([128, 512], F32, tag="sh", name="po_t")
                                  for p in range(PAIRS):
                                      nc.tensor.matmul(
                                          po[:],
                                          A[p][:, 128 * tl : 128 * (tl + 1)],
                                          wo_t[p][:, 512 * eh : 512 * (eh + 1)],
                                          start=(p == 0),
                                          stop=(p == PAIRS - 1),
                                      )
                                  ob = osbp.tile([128, 512], BF16, tag="ob", name="ob_t")
                                  nc.vector.tensor_copy(ob[:], po[:])
                                  r0 = 512 * t + 128 * tl
                                  nc.sync.dma_start(
                                      outp.ap()[r0 : r0 + 128, 512 * eh : 512 * (eh + 1)],
                                      ob[:],
                                  )
                              normal.append((t, oproj))

                  for q in (urgent, projq, lateq):
                      while q:
                          q.pop(0)()
                  while normal:
                      normal.pop(0)[1]()

    nc.compile()
    return nc


_RT = {}


def _get_runtime():
    if "rt" in _RT:
        return _RT["rt"]

    import jax
    import numpy as np
    from jax.experimental.shard_map import shard_map
    from jax.sharding import Mesh, PartitionSpec

    import concourse.mybir as mybir
    from concourse.bass2jax import (
        _bass_exec_p,
        install_neuronx_cc_hook,
        partition_id_tensor,
    )

    nc = _build_nc()
    install_neuronx_cc_hook()

    partition_name = nc.partition_id_tensor.name if nc.partition_id_tensor else None
    in_names, out_names, out_avals, zero_shapes = [], [], [], []
    for alloc in nc.m.functions[0].allocations:
        if not isinstance(alloc, mybir.MemoryLocationSet):
            continue
        if not alloc.memorylocations:
            continue
        name = alloc.memorylocations[0].name
        if alloc.kind == "ExternalInput":
            if name != partition_name:
                in_names.append(name)
        elif alloc.kind == "ExternalOutput":
            shape = tuple(alloc.tensor_shape)
            dtype = mybir.dt.np(alloc.dtype)
            out_names.append(name)
            out_avals.append(jax.core.ShapedArray(shape, dtype))
            zero_shapes.append((shape, dtype))
    n_params = len(in_names)
    n_outs = len(out_names)
    all_in_names = list(in_names) + list(out_names)
    if partition_name is not None:
        all_in_names.append(partition_name)
    donate = tuple(range(n_params, n_params + n_outs))

    def _body(*args):
        operands = list(args)
        if partition_name is not None:
            operands.append(partition_id_tensor())
        outs = _bass_exec_p.bind(
            *operands,
            out_avals=tuple(out_avals),
            in_names=tuple(all_in_names),
            out_names=tuple(out_names),
            lowering_input_output_aliases=(),
            sim_require_finite=True,
            sim_require_nnan=True,
            nc=nc,
        )
        return tuple(outs)

    devices = jax.devices()[:NCORES]
    assert len(devices) == NCORES
    mesh = Mesh(np.asarray(devices), ("core",))
    in_specs = (PartitionSpec("core"),) * (n_params + n_outs)
    out_specs = (PartitionSpec("core"),) * n_outs
    fn = jax.jit(
        shard_map(_body, mesh=mesh, in_specs=in_specs, out_specs=out_specs,
                  check_rep=False),
        donate_argnums=donate,
        keep_unused=True,
    )
    rt = {
        "fn": fn,
        "in_names": in_names,
        "out_names": out_names,
        "zero_shapes": zero_shapes,
        "n_params": n_params,
        "mesh": mesh,
        "nc": nc,
    }
    _RT["rt"] = rt
    return rt


def _make_masks():
    kk = np.arange(128, dtype=np.int64)[:, None]
    q = np.arange(512, dtype=np.int64)[None, :]
    masks = np.zeros((4, 128, 1024), dtype=np.float32)
    for m in range(4):
        half = ((128 * m + kk) <= q).astype(np.float32)
        masks[m, :, 0:512] = half
        masks[m, :, 512:1024] = half
    return masks


def _shard_inputs(query, key, value, Wq, bq, Wk, bk, Wv, bv, Wo, bo, pad_mask):
    f = np.float32
    import ml_dtypes
    bf = ml_dtypes.bfloat16
    query = np.asarray(query, f).reshape(B, S, D)
    key = np.asarray(key, f).reshape(B, S, D)
    value = np.asarray(value, f).reshape(B, S, D)
    consts = {
        "onescol": np.ones((1, 128), bf),
        "selp": np.ones((65, 64), f),
        "masks": _make_masks().astype(bf),
    }
    xT = {b: {
        "xqT": query[b].T.astype(bf),
        "xkT": key[b].T.astype(bf),
        "xvT": value[b].T.astype(bf),
    } for b in range(B)}
    wT = {
        "q": np.asarray(Wq, f).T,
        "k": np.asarray(Wk, f).T,
        "v": np.asarray(Wv, f).T,
        "o": np.asarray(Wo, f).T,
    }
    in_maps = []
    for c in range(NCORES):
        b = c // 2
        hh = c % 2
        sl = slice(DH * hh, DH * (hh + 1))
        m = {
            **xT[b],
            "wqt": wT["q"][:, sl].astype(bf),
            "wkt": wT["k"][:, sl].astype(bf),
            "wvt": wT["v"][:, sl].astype(bf),
            "wot": wT["o"][sl, :].astype(bf),
            "bq": np.ascontiguousarray(np.asarray(bq, f)[sl]),
            "bk": np.ascontiguousarray(np.asarray(bk, f)[sl]),
            "bv": np.asarray(bv, f)[sl].reshape(1, DH).astype(bf),
            **consts,
        }
        in_maps.append(m)
    return in_maps


def _run(rt, in_maps):
    import jax
    import numpy as np

    n = rt["n_params"]
    concat_in = [
        np.concatenate([np.asarray(in_maps[c][name]) for c in range(NCORES)], axis=0)
        for name in rt["in_names"]
    ]
    concat_zeros = [
        np.zeros((NCORES * sh[0], *sh[1:]), dt) for sh, dt in rt["zero_shapes"]
    ]
    out_arrs = rt["fn"](*concat_in, *concat_zeros)
    res = []
    for c in range(NCORES):
        d = {}
        for i, name in enumerate(rt["out_names"]):
            sh = rt["zero_shapes"][i][0]
            d[name] = np.asarray(out_arrs[i]).reshape(NCORES, *sh)[c]
        res.append(d)
    return res


def kernel(**inputs):
    rt = _get_runtime()
    in_maps = _shard_inputs(**inputs)
    res = _run(rt, in_maps)
    bo = np.asarray(inputs["bo"], np.float32)
    out = np.empty((B, S, D), dtype=np.float32)
    for b in range(B):
        out[b] = (
            np.asarray(res[2 * b]["outp"], np.float32)
            + np.asarray(res[2 * b + 1]["outp"], np.float32)
            + bo
        )
    return out



# revision 16
# speedup vs baseline: 5.4432x; 5.4432x over previous
"""Trainium2 Bass kernel for nn_MultiHeadAttention (B=4, S=2048, D=1024, H=16, causal).

Sharding: 8 cores = 4 batches x 2 head-halves (8 heads each). Every core runs an
identical SPMD program: Q/K/V projections for its 8 heads over its batch's 2048
tokens, causal flash-attention, and a partial output projection over its 512
head-dims. Host unshard adds the two partial outputs per batch (+ bo).

All matmul operands are bf16 (fp32 PSUM accumulation). The softmax denominator
is fused into the PV matmul via a ones-column appended to each head's V block
(V tiles are [128 tok, 8 heads, 96]: cols 0-63 = V, col 64 = 1.0, rest 0 —
matmul output partition counts must be multiples of 32). Denominators are
reciprocal'd in place (DVE ops cannot shift partitions downward), broadcast to
all partitions via K=1 matmuls, and applied with scalar_tensor_tensor.
Diagonal-crossing k-chunks narrow their score/exp/PV ops to the unmasked
q-range. Projections for q-tile t+1, pair tails, and the out-projection of
tile t-1 are interleaved into tile t's chunk stream through a priority work
queue so the PE stays fed while the ACT engine grinds exp (the per-chunk
critical resource). Weight/x loads are single rearranged DMAs ([1024,512] ->
[128,8,512]) to cut HWDGE descriptor cost; attention-phase constants load
behind the first projection tiles.
"""

import os
import sys

for _p in ("/opt/trn_rl_repo", "/root/.axon_site/_ro/trn_rl_repo"):
    if os.path.isdir(_p) and _p not in sys.path:
        sys.path.insert(0, _p)

import numpy as np

B, S, D, H = 4, 2048, 1024, 16
HD = D // H  # 64
DH = D // 2  # 512 dims per head-half
NCORES = 8
QT_TILES = 4      # 512-token q tiles
PAIRS = 4         # head pairs per core (8 heads)
ICHUNKS = 8       # 128-row feature chunks of D
TT16 = 16         # 128-token tiles


def _build_nc(repeat=1):
    import concourse.mybir as mybir
    import concourse.tile as tile
    from concourse import bacc

    F32 = mybir.dt.float32
    F32R = mybir.dt.float32r
    BF16 = mybir.dt.bfloat16
    ACTF = mybir.ActivationFunctionType
    ALU = mybir.AluOpType

    nc = bacc.Bacc("TRN2", target_bir_lowering=False, debug=False, num_devices=NCORES)

    xqT = nc.dram_tensor("xqT", [D, S], BF16, kind="ExternalInput")
    xkT = nc.dram_tensor("xkT", [D, S], BF16, kind="ExternalInput")
    xvT = nc.dram_tensor("xvT", [D, S], BF16, kind="ExternalInput")
    wqt = nc.dram_tensor("wqt", [D, DH], BF16, kind="ExternalInput")
    wkt = nc.dram_tensor("wkt", [D, DH], BF16, kind="ExternalInput")
    wvt = nc.dram_tensor("wvt", [D, DH], BF16, kind="ExternalInput")
    wot = nc.dram_tensor("wot", [DH, D], BF16, kind="ExternalInput")
    bq = nc.dram_tensor("bq", [DH], F32, kind="ExternalInput")
    bk = nc.dram_tensor("bk", [DH], F32, kind="ExternalInput")
    bv = nc.dram_tensor("bv", [1, DH], BF16, kind="ExternalInput")
    onescol = nc.dram_tensor("onescol", [1, 128], BF16, kind="ExternalInput")
    seld = nc.dram_tensor("selp", [65, 64], F32R, kind="ExternalInput")
    # single causal-mask base: mask_m[r, c] == maskb[r, c + 384 - 128*m]
    masksd = nc.dram_tensor("masks", [128, 896], BF16, kind="ExternalInput")
    outp = nc.dram_tensor("outp", [S, D], BF16, kind="ExternalOutput")

    with tile.TileContext(nc) as tc:
        with (
            tc.tile_pool(name="const", bufs=1) as cp,
            tc.tile_pool(name="persist", bufs=1) as pp,
        ):
            onescol_t = cp.tile([1, 128], BF16, tag="onescol", name="onescol_t")
            selp_t = cp.tile([65, 64], F32R, tag="selp", name="selp_t")
            bv_t = cp.tile([1, DH], BF16, tag="bv", name="bv_t")
            mbase = cp.tile([128, 896], BF16, tag="maskb", name="maskb_t")

            def mask_ap(m, c0, c1):
                # [128, c1-c0] causal mask slice for 128-row chunk offset m
                off = 384 - 128 * m
                return mbase[:, off + c0 : off + c1]

            bq_t, bk_t = [], []
            for p in range(PAIRS):
                bq_t.append(cp.tile([128, 1], F32, tag=f"bq{p}", name=f"bq_t{p}"))
                bk_t.append(cp.tile([128, 1], F32, tag=f"bk{p}", name=f"bk_t{p}"))

            def load_consts_early():
                nc.sync.dma_start(onescol_t[:], onescol.ap())
                nc.sync.dma_start(bv_t[:], bv.ap())
                for p in range(PAIRS):
                    nc.sync.dma_start(bq_t[p][:], bq.ap()[128 * p : 128 * (p + 1)])
                    nc.sync.dma_start(bk_t[p][:], bk.ap()[128 * p : 128 * (p + 1)])

            def load_consts_mid():
                # mask needed by the very first (diagonal) chunk
                nc.sync.dma_start(mbase[:], masksd.ap())

            def load_consts_late():
                # attention-phase consts: behind all projection loads
                nc.sync.dma_start(selp_t[:], seld.ap())
                for p in range(PAIRS):
                    nc.sync.dma_start(wo_t[p][:], wot.ap()[128 * p : 128 * (p + 1), :])
            # wo resident: 4 chunk tiles [128 d, 1024 e] (DMA deferred past first proj loads)
            wo_t = []
            for p in range(PAIRS):
                wt = pp.tile([128, D], BF16, tag=f"wo{p}", name=f"wo_t{p}")
                wo_t.append(wt)

            # persistent activations (feature-major Q/K; token-major V)
            QT = [pp.tile([128, S], BF16, tag=f"qt{p}", name=f"QT{p}") for p in range(PAIRS)]
            KT = [pp.tile([128, S], BF16, tag=f"kt{p}", name=f"KT{p}") for p in range(PAIRS)]
            # V: [128 tok, 8 head-groups, 128]; col 64 of each group stays 1.0.
            # 128-wide weight slices turn on FWL (fast weight load) for PV matmuls.
            V = [pp.tile([128, 8, 128], BF16, tag=f"v{i}", name=f"V{i}") for i in range(TT16)]
            for i in range(TT16):
                nc.vector.memset(V[i][:, :, 64:128], 0.0)
                nc.vector.memset(V[i][:, :, 64:65], 1.0)

            for _rep in range(repeat):
              with (
                  tc.tile_pool(name="xtb", bufs=4) as xtbp,
                  tc.tile_pool(name="wp", bufs=3) as wp,
                  tc.tile_pool(name="pbp", bufs=6) as pbp,
                  tc.tile_pool(name="rpool", bufs=3) as rpool,
                  tc.tile_pool(name="apool", bufs=3) as apool,
                  tc.tile_pool(name="osb", bufs=4) as osbp,
                  tc.tile_pool(name="sps", bufs=2, space="PSUM") as sps,
                  tc.tile_pool(name="acc", bufs=2, space="PSUM") as accps,
                  tc.tile_pool(name="shp", bufs=2, space="PSUM") as shps,
              ):
                  urgent, projq, lateq, normal = [], [], [], []
                  pace = {"iter": 0, "total": 1, "next_norm": 0}

                  def drain_one():
                      # urgent (softmax tails), proj prerequisites, and late
                      # K/V groups drain immediately; out-proj units are spaced
                      # across the tile so late ACT-bound chunks keep PE fed
                      for q in (urgent, projq, lateq):
                          if q:
                              q.pop(0)()
                              return
                      if normal and pace["iter"] >= pace["next_norm"]:
                          step = max(1, pace["total"] // 10)
                          pace["next_norm"] = pace["iter"] + step
                          normal.pop(0)[1]()

                  def load_w(wdram, split=False):
                      # one DMA: dram [(c p), q] -> sbuf [p, c, q]
                      wt = wp.tile([128, ICHUNKS, DH], BF16, tag="w", name="w_t")
                      src = wdram.ap().rearrange("(c p) q -> p c q", c=ICHUNKS)
                      if split:
                          # chunk 0 lands first so the first proj matmul can start
                          nc.sync.dma_start(wt[:, 0:2, :], src[:, 0:2, :])
                          nc.sync.dma_start(wt[:, 2:ICHUNKS, :], src[:, 2:ICHUNKS, :])
                      else:
                          nc.sync.dma_start(wt[:], src)
                      return wt

                  def dma_x(xdram, t, split=False):
                      tsl = slice(512 * t, 512 * (t + 1))
                      xt = xtbp.tile([128, ICHUNKS, 512], BF16, tag="xtb", name="xtb_tile")
                      src = xdram.ap()[:, tsl].rearrange("(c p) q -> p c q", c=ICHUNKS)
                      if split:
                          nc.sync.dma_start(xt[:, 0:2, :], src[:, 0:2, :])
                          nc.sync.dma_start(xt[:, 2:ICHUNKS, :], src[:, 2:ICHUNKS, :])
                      else:
                          nc.sync.dma_start(xt[:], src)
                      return xt

                  def qk_group(w_t, xtb, dst, bias_tiles, t, p, on_act=False):
                      pg = shps.tile([128, 512], F32, tag="sh", name="pg_t")
                      for c in range(ICHUNKS):
                          nc.tensor.matmul(
                              pg[:], w_t[:, c, 128 * p : 128 * (p + 1)], xtb[:, c, :],
                              start=(c == 0), stop=(c == ICHUNKS - 1),
                          )
                      if on_act:
                          # PSUM->SBUF + per-partition bias on the ACT engine
                          # (identity shares the exp activation table: no reload)
                          nc.scalar.activation(
                              dst[p][:, 512 * t : 512 * (t + 1)], pg[:],
                              ACTF.Identity, bias=bias_tiles[p][:], scale=1.0)
                      else:
                          nc.vector.tensor_scalar_add(
                              dst[p][:, 512 * t : 512 * (t + 1)], pg[:], bias_tiles[p][:])

                  def v_group(wv_t, xtb, t, b):
                      pg = shps.tile([128, 512], F32, tag="sh", name="pg_t")
                      for c in range(ICHUNKS):
                          nc.tensor.matmul(
                              pg[:], xtb[:, c, 128 * b : 128 * (b + 1)], wv_t[:, c, :],
                              start=(c == 0), stop=(c == ICHUNKS - 1),
                          )
                      nc.vector.scalar_tensor_tensor(
                          V[4 * t + b][:, :, 0:64], pg[:], 1.0, bvb[:],
                          ALU.mult, ALU.add,
                      )

                  # ---- startup: K weights + K x-tile first, consts behind ----
                  wk_t = load_w(wkt, split=(_rep == 0))
                  xk = dma_x(xkT, 0, split=(_rep == 0))
                  if _rep == 0:
                      load_consts_early()
                  wq_t = load_w(wqt, split=(_rep == 0))
                  xq = dma_x(xqT, 0, split=(_rep == 0))
                  if _rep == 0:
                      load_consts_mid()
                  wv_t = load_w(wvt, split=(_rep == 0))
                  xv = dma_x(xvT, 0, split=(_rep == 0))
                  if _rep == 0:
                      load_consts_late()

                  # ---- tile-0 projections emitted directly ----
                  for p in range(PAIRS):
                      qk_group(wk_t, xk, KT, bk_t, 0, p, on_act=True)
                  for p in range(PAIRS):
                      qk_group(wq_t, xq, QT, bq_t, 0, p)
                  # bv broadcast tile [128, 512]
                  pbv = shps.tile([128, 512], F32, tag="sh", name="pbv_tile")
                  nc.tensor.matmul(pbv[:], onescol_t[:], bv_t[:], start=True, stop=True)
                  bvb = wp.tile([128, DH], BF16, tag="bvb", name="bvb_tile")
                  nc.vector.tensor_copy(bvb[:], pbv[:])
                  for b in range(4):
                      v_group(wv_t, xv, 0, b)

                  def queue_proj(t1):
                      st = {}
                      kv = []
                      def dq(): st["xq"] = dma_x(xqT, t1)
                      projq.append(dq)
                      for p in range(PAIRS):
                          projq.append(lambda p=p: qk_group(wq_t, st["xq"], QT, bq_t, t1, p))
                      def dk(): st["xk"] = dma_x(xkT, t1)
                      kv.append(dk)
                      for p in range(PAIRS):
                          kv.append(lambda p=p: qk_group(
                              wk_t, st["xk"], KT, bk_t, t1, p, on_act=True))
                      def dv(): st["xv"] = dma_x(xvT, t1)
                      kv.append(dv)
                      for b in range(4):
                          kv.append(lambda b=b: v_group(wv_t, st["xv"], t1, b))
                      if t1 == QT_TILES - 1:
                          # last tile: its own chunks only touch K/V tile t1 from
                          # chunk j=4*t1 (iter 13) on; safe to drain in-stream
                          lateq.extend(kv)
                      else:
                          projq.extend(kv)

                  # ---- attention with interleaved proj/out-proj ----
                  for t in range(QT_TILES):
                      while projq:       # proj for this tile must be emitted
                          projq.pop(0)()
                      while normal and normal[0][0] <= t - 2:
                          # out-proj units two tiles back must emit before this
                          # tile's pair tails reuse their `a` buffers
                          normal.pop(0)[1]()
                      if t < QT_TILES - 1:
                          queue_proj(t + 1)
                      nch = 4 * (t + 1)
                      pace["iter"], pace["total"], pace["next_norm"] = 0, 4 * nch, 0
                      A = []
                      for p in range(PAIRS):
                          psA = accps.tile([128, 512], F32, tag="acc", name="psA_t")
                          psB = accps.tile([128, 512], F32, tag="acc", name="psB_t")
                          for j in range(nch):
                              ksl = slice(128 * j, 128 * (j + 1))
                              m = j - 4 * t
                              q0 = 128 * m if m >= 1 else 0
                              s01 = sps.tile([128, 1024], F32, tag="s01", name="s01_t")
                              nc.tensor.matmul(
                                  s01[:, q0:512], KT[p][0:64, ksl],
                                  QT[p][0:64, 512 * t + q0 : 512 * (t + 1)],
                                  start=True, stop=True,
                              )
                              nc.tensor.matmul(
                                  s01[:, 512 + q0 : 1024], KT[p][64:128, ksl],
                                  QT[p][64:128, 512 * t + q0 : 512 * (t + 1)],
                                  start=True, stop=True,
                              )
                              pb = pbp.tile([128, 1024], BF16, tag="pb", name="pb_t")
                              if q0 == 0:
                                  nc.scalar.activation(pb[:], s01[:], ACTF.Exp, scale=0.125)
                              else:
                                  nc.scalar.activation(
                                      pb[:, q0:512], s01[:, q0:512], ACTF.Exp, scale=0.125)
                                  nc.scalar.activation(
                                      pb[:, 512 + q0 : 1024], s01[:, 512 + q0 : 1024],
                                      ACTF.Exp, scale=0.125)
                              if m >= 0:
                                  msk = mask_ap(m, q0, 512)
                                  nc.vector.tensor_tensor(
                                      pb[:, q0:512], pb[:, q0:512], msk, ALU.mult)
                                  nc.vector.tensor_tensor(
                                      pb[:, 512 + q0 : 1024], pb[:, 512 + q0 : 1024],
                                      msk, ALU.mult)
                              st_ = (j == 0)
                              sp_ = (j == nch - 1)
                              nc.tensor.matmul(
                                  psA[:, q0:512], V[j][:, 2 * p : 2 * p + 1, :],
                                  pb[:, q0:512], start=st_, stop=sp_,
                              )
                              nc.tensor.matmul(
                                  psB[:, q0:512], V[j][:, 2 * p + 1 : 2 * p + 2, :],
                                  pb[:, 512 + q0 : 1024], start=st_, stop=sp_,
                              )
                              pace["iter"] += 1
                              drain_one()

                          def pair_tail(p=p, psA=psA, psB=psB, A=A):
                              r2 = rpool.tile([65, 512], F32R, tag="r", name="r_t")
                              r2b = rpool.tile([65, 512], F32R, tag="r", name="r2b_t")
                              with nc.allow_low_precision(reason="f32r storage is fp32"):
                                  nc.vector.reciprocal(r2[64:65, :], psA[64:65, :])
                                  nc.vector.reciprocal(r2b[64:65, :], psB[64:65, :])
                              rbA = shps.tile([64, 512], F32, tag="sh", name="rbA_t")
                              rbB = shps.tile([64, 512], F32, tag="sh", name="rbB_t")
                              nc.tensor.matmul(rbA[:], selp_t[64:65, :], r2[64:65, :],
                                               start=True, stop=True)
                              nc.tensor.matmul(rbB[:], selp_t[64:65, :], r2b[64:65, :],
                                               start=True, stop=True)
                              rbc = rpool.tile([128, 512], BF16, tag="rbc", name="rbc_t")
                              nc.vector.tensor_copy(rbc[0:64, :], rbA[:])
                              nc.vector.tensor_scalar_mul(rbc[64:128, :], rbB[:], 1.0)
                              a = apool.tile([128, 512], BF16, tag=f"a{p}", name=f"a_t{p}")
                              nc.vector.scalar_tensor_tensor(
                                  a[0:64, :], psA[0:64, :], 1.0, rbc[0:64, :],
                                  ALU.mult, ALU.mult,
                              )
                              nc.vector.scalar_tensor_tensor(
                                  a[64:128, :], psB[0:64, :], 1.0, rbc[64:128, :],
                                  ALU.mult, ALU.mult,
                              )
                              A.append(a)

                          urgent.append(pair_tail)

                      for tl in range(4):
                          for eh in range(2):
                              def oproj(t=t, tl=tl, eh=eh, A=A):
                                  po = shps.tile([128, 512], F32, tag="sh", name="po_t")
                                  for p in range(PAIRS):
                                      nc.tensor.matmul(
                                          po[:],
                                          A[p][:, 128 * tl : 128 * (tl + 1)],
                                          wo_t[p][:, 512 * eh : 512 * (eh + 1)],
                                          start=(p == 0),
                                          stop=(p == PAIRS - 1),
                                      )
                                  ob = osbp.tile([128, 512], BF16, tag="ob", name="ob_t")
                                  nc.vector.tensor_copy(ob[:], po[:])
                                  r0 = 512 * t + 128 * tl
                                  nc.sync.dma_start(
                                      outp.ap()[r0 : r0 + 128, 512 * eh : 512 * (eh + 1)],
                                      ob[:],
                                  )
                              normal.append((t, oproj))

                  for q in (urgent, projq, lateq):
                      while q:
                          q.pop(0)()
                  while normal:
                      normal.pop(0)[1]()

    nc.compile()
    return nc


_RT = {}


def _get_runtime():
    if "rt" in _RT:
        return _RT["rt"]

    import jax
    import numpy as np
    from jax.experimental.shard_map import shard_map
    from jax.sharding import Mesh, PartitionSpec

    import concourse.mybir as mybir
    from concourse.bass2jax import (
        _bass_exec_p,
        install_neuronx_cc_hook,
        partition_id_tensor,
    )

    nc = _build_nc()
    install_neuronx_cc_hook()

    partition_name = nc.partition_id_tensor.name if nc.partition_id_tensor else None
    in_names, out_names, out_avals, zero_shapes = [], [], [], []
    for alloc in nc.m.functions[0].allocations:
        if not isinstance(alloc, mybir.MemoryLocationSet):
            continue
        if not alloc.memorylocations:
            continue
        name = alloc.memorylocations[0].name
        if alloc.kind == "ExternalInput":
            if name != partition_name:
                in_names.append(name)
        elif alloc.kind == "ExternalOutput":
            shape = tuple(alloc.tensor_shape)
            dtype = mybir.dt.np(alloc.dtype)
            out_names.append(name)
            out_avals.append(jax.core.ShapedArray(shape, dtype))
            zero_shapes.append((shape, dtype))
    n_params = len(in_names)
    n_outs = len(out_names)
    all_in_names = list(in_names) + list(out_names)
    if partition_name is not None:
        all_in_names.append(partition_name)
    donate = tuple(range(n_params, n_params + n_outs))

    def _body(*args):
        operands = list(args)
        if partition_name is not None:
            operands.append(partition_id_tensor())
        outs = _bass_exec_p.bind(
            *operands,
            out_avals=tuple(out_avals),
            in_names=tuple(all_in_names),
            out_names=tuple(out_names),
            lowering_input_output_aliases=(),
            sim_require_finite=True,
            sim_require_nnan=True,
            nc=nc,
        )
        return tuple(outs)

    devices = jax.devices()[:NCORES]
    assert len(devices) == NCORES
    mesh = Mesh(np.asarray(devices), ("core",))
    in_specs = (PartitionSpec("core"),) * (n_params + n_outs)
    out_specs = (PartitionSpec("core"),) * n_outs
    fn = jax.jit(
        shard_map(_body, mesh=mesh, in_specs=in_specs, out_specs=out_specs,
                  check_rep=False),
        donate_argnums=donate,
        keep_unused=True,
    )
    rt = {
        "fn": fn,
        "in_names": in_names,
        "out_names": out_names,
        "zero_shapes": zero_shapes,
        "n_params": n_params,
        "mesh": mesh,
        "nc": nc,
    }
    _RT["rt"] = rt
    return rt


def _make_masks():
    # causal base: maskb[r, u] = 1[u >= r + 384]; mask_m = maskb[:, 384-128m:...]
    r = np.arange(128, dtype=np.int64)[:, None]
    u = np.arange(896, dtype=np.int64)[None, :]
    return (u >= r + 384).astype(np.float32)


def _shard_inputs(query, key, value, Wq, bq, Wk, bk, Wv, bv, Wo, bo, pad_mask):
    f = np.float32
    import ml_dtypes
    bf = ml_dtypes.bfloat16
    query = np.asarray(query, f).reshape(B, S, D)
    key = np.asarray(key, f).reshape(B, S, D)
    value = np.asarray(value, f).reshape(B, S, D)
    consts = {
        "onescol": np.ones((1, 128), bf),
        "selp": np.ones((65, 64), f),
        "masks": _make_masks().astype(bf),
    }
    xT = {b: {
        "xqT": query[b].T.astype(bf),
        "xkT": key[b].T.astype(bf),
        "xvT": value[b].T.astype(bf),
    } for b in range(B)}
    wT = {
        "q": np.asarray(Wq, f).T,
        "k": np.asarray(Wk, f).T,
        "v": np.asarray(Wv, f).T,
        "o": np.asarray(Wo, f).T,
    }
    in_maps = []
    for c in range(NCORES):
        b = c // 2
        hh = c % 2
        sl = slice(DH * hh, DH * (hh + 1))
        m = {
            **xT[b],
            "wqt": wT["q"][:, sl].astype(bf),
            "wkt": wT["k"][:, sl].astype(bf),
            "wvt": wT["v"][:, sl].astype(bf),
            "wot": wT["o"][sl, :].astype(bf),
            "bq": np.ascontiguousarray(np.asarray(bq, f)[sl]),
            "bk": np.ascontiguousarray(np.asarray(bk, f)[sl]),
            "bv": np.asarray(bv, f)[sl].reshape(1, DH).astype(bf),
            **consts,
        }
        in_maps.append(m)
    return in_maps


def _run(rt, in_maps):
    import jax
    import numpy as np

    n = rt["n_params"]
    concat_in = [
        np.concatenate([np.asarray(in_maps[c][name]) for c in range(NCORES)], axis=0)
        for name in rt["in_names"]
    ]
    concat_zeros = [
        np.zeros((NCORES * sh[0], *sh[1:]), dt) for sh, dt in rt["zero_shapes"]
    ]
    out_arrs = rt["fn"](*concat_in, *concat_zeros)
    res = []
    for c in range(NCORES):
        d = {}
        for i, name in enumerate(rt["out_names"]):
            sh = rt["zero_shapes"][i][0]
            d[name] = np.asarray(out_arrs[i]).reshape(NCORES, *sh)[c]
        res.append(d)
    return res


def kernel(**inputs):
    rt = _get_runtime()
    in_maps = _shard_inputs(**inputs)
    res = _run(rt, in_maps)
    bo = np.asarray(inputs["bo"], np.float32)
    out = np.empty((B, S, D), dtype=np.float32)
    for b in range(B):
        out[b] = (
            np.asarray(res[2 * b]["outp"], np.float32)
            + np.asarray(res[2 * b + 1]["outp"], np.float32)
            + bo
        )
    return out



# revision 36
# speedup vs baseline: 6.7647x; 1.2428x over previous
"""Trainium2 Bass kernel for nn_MultiHeadAttention (B=4, S=2048, D=1024, H=16, causal).

Sharding: 8 cores = 4 batches x 2 head-halves (8 heads each). Every core runs an
identical SPMD program: Q/K/V projections for its 8 heads over its batch's 2048
tokens, causal flash-attention, and a partial output projection over its 512
head-dims. Host unshard adds the two partial outputs per batch (+ bo).

All matmul operands are bf16 (fp32 PSUM accumulation). The softmax denominator
is fused into the PV matmul via a ones-column appended to each head's V block
(V tiles are [128 tok, 8 heads, 96]: cols 0-63 = V, col 64 = 1.0, rest 0 —
matmul output partition counts must be multiples of 32). Denominators are
reciprocal'd in place (DVE ops cannot shift partitions downward), broadcast to
all partitions via K=1 matmuls, and applied with scalar_tensor_tensor.
Diagonal-crossing k-chunks narrow their score/exp/PV ops to the unmasked
q-range. Projections for q-tile t+1, pair tails, and the out-projection of
tile t-1 are interleaved into tile t's chunk stream through a priority work
queue so the PE stays fed while the ACT engine grinds exp (the per-chunk
critical resource). Weight/x loads are single rearranged DMAs ([1024,512] ->
[128,8,512]) to cut HWDGE descriptor cost; attention-phase constants load
behind the first projection tiles.
"""

import os
import sys

for _p in ("/opt/trn_rl_repo", "/root/.axon_site/_ro/trn_rl_repo"):
    if os.path.isdir(_p) and _p not in sys.path:
        sys.path.insert(0, _p)

import numpy as np

B, S, D, H = 4, 2048, 1024, 16
HD = D // H  # 64
DH = D // 2  # 512 dims per head-half
NCORES = 8
QT_TILES = 4      # 512-token q tiles
PAIRS = 4         # head pairs per core (8 heads)
ICHUNKS = 8       # 128-row feature chunks of D
TT16 = 16         # 128-token tiles


def _build_nc(repeat=1):
    import concourse.mybir as mybir
    import concourse.tile as tile
    from concourse import bacc

    F32 = mybir.dt.float32
    F32R = mybir.dt.float32r
    BF16 = mybir.dt.bfloat16
    ACTF = mybir.ActivationFunctionType
    ALU = mybir.AluOpType

    nc = bacc.Bacc("TRN2", target_bir_lowering=False, debug=False, num_devices=NCORES)

    xqT = nc.dram_tensor("xqT", [D, S], BF16, kind="ExternalInput")
    xkT = nc.dram_tensor("xkT", [D, S], BF16, kind="ExternalInput")
    xvT = nc.dram_tensor("xvT", [D, S], BF16, kind="ExternalInput")
    # weights host-prearranged to [128, c, q] so DMA lines are 8 KB contiguous
    wqt = nc.dram_tensor("wqt", [128, ICHUNKS * DH], BF16, kind="ExternalInput")
    wkt = nc.dram_tensor("wkt", [128, ICHUNKS * DH], BF16, kind="ExternalInput")
    wvt = nc.dram_tensor("wvt", [128, ICHUNKS * DH], BF16, kind="ExternalInput")
    wot = nc.dram_tensor("wot", [DH, D], BF16, kind="ExternalInput")
    # packed per-partition biases: col 0:4 = bq pairs, 4:8 = bk pairs
    bqk = nc.dram_tensor("bqk", [128, 8], F32, kind="ExternalInput")
    bv = nc.dram_tensor("bv", [1, DH], BF16, kind="ExternalInput")
    onescol = nc.dram_tensor("onescol", [1, 128], BF16, kind="ExternalInput")
    seld = nc.dram_tensor("selp", [66, 128], F32R, kind="ExternalInput")
    # single causal-mask base: mask_m[r, c] == maskb[r, c + 384 - 128*m]
    masksd = nc.dram_tensor("masks", [128, 896], BF16, kind="ExternalInput")
    outp = nc.dram_tensor("outp", [S, D], BF16, kind="ExternalOutput")

    with tile.TileContext(nc) as tc:
        with (
            tc.tile_pool(name="const", bufs=1) as cp,
            tc.tile_pool(name="persist", bufs=1) as pp,
        ):
            onescol_t = cp.tile([1, 128], BF16, tag="onescol", name="onescol_t")
            selp_t = cp.tile([66, 128], F32R, tag="selp", name="selp_t")
            bv_t = cp.tile([1, DH], BF16, tag="bv", name="bv_t")
            mbase = cp.tile([128, 896], BF16, tag="maskb", name="maskb_t")

            def mask_ap(m, c0, c1):
                # [128, c1-c0] causal mask slice for 128-row chunk offset m
                off = 384 - 128 * m
                return mbase[:, off + c0 : off + c1]

            bqk_t = cp.tile([128, 8], F32, tag="bqk", name="bqk_t")
            bq_t = [bqk_t[:, p : p + 1] for p in range(PAIRS)]
            bk_t = [bqk_t[:, 4 + p : 5 + p] for p in range(PAIRS)]

            def load_consts_early():
                nc.sync.dma_start(onescol_t[:], onescol.ap())
                nc.sync.dma_start(bv_t[:], bv.ap())
                nc.sync.dma_start(bqk_t[:], bqk.ap())

            def load_consts_mid():
                # mask needed by the very first (diagonal) chunk
                nc.sync.dma_start(mbase[:], masksd.ap())

            def load_consts_late():
                # attention-phase consts: behind all projection loads
                nc.sync.dma_start(selp_t[:], seld.ap())
                for p in range(PAIRS):
                    nc.sync.dma_start(wo_t[p][:], wot.ap()[128 * p : 128 * (p + 1), :])
            # wo resident: 4 chunk tiles [128 d, 1024 e] (DMA deferred past first proj loads)
            wo_t = []
            for p in range(PAIRS):
                wt = pp.tile([128, D], BF16, tag=f"wo{p}", name=f"wo_t{p}")
                wo_t.append(wt)

            # persistent activations (feature-major Q/K; token-major V)
            QT = [pp.tile([128, S], BF16, tag=f"qt{p}", name=f"QT{p}") for p in range(PAIRS)]
            KT = [pp.tile([128, S], BF16, tag=f"kt{p}", name=f"KT{p}") for p in range(PAIRS)]
            # V: [128 tok, 8 head-groups, 128]; col 64 of each group stays 1.0.
            # 128-wide weight slices turn on FWL (fast weight load) for PV matmuls.
            V = [pp.tile([128, 8, 128], BF16, tag=f"v{i}", name=f"V{i}") for i in range(TT16)]
            for i in range(TT16):
                nc.vector.memset(V[i][:, :, 64:128], 0.0)
                nc.vector.memset(V[i][:, :, 64:65], 1.0)

            for _rep in range(repeat):
              with (
                  tc.tile_pool(name="xtb", bufs=4) as xtbp,
                  tc.tile_pool(name="wp", bufs=3) as wp,
                  tc.tile_pool(name="pbp", bufs=6) as pbp,
                  tc.tile_pool(name="rpool", bufs=3) as rpool,
                  tc.tile_pool(name="apool", bufs=3) as apool,
                  tc.tile_pool(name="osb", bufs=4) as osbp,
                  tc.tile_pool(name="sps", bufs=2, space="PSUM") as sps,
                  tc.tile_pool(name="acc", bufs=2, space="PSUM") as accps,
                  tc.tile_pool(name="shp", bufs=2, space="PSUM") as shps,
              ):
                  urgent, projq, lateq, normal = [], [], [], []
                  pace = {"iter": 0, "total": 1, "next_norm": 0}

                  def drain_one():
                      # urgent (softmax tails), proj prerequisites, and late
                      # K/V groups drain immediately; out-proj units are spaced
                      # across the tile so late ACT-bound chunks keep PE fed
                      for q in (urgent, projq, lateq):
                          if q:
                              q.pop(0)()
                              return
                      if normal and pace["iter"] >= pace["next_norm"]:
                          step = max(1, pace["total"] // 10)
                          pace["next_norm"] = pace["iter"] + step
                          normal.pop(0)[1]()

                  def load_w(wdram, split=False):
                      # host-prearranged: one DMA with 8 KB/partition lines
                      wt = wp.tile([128, ICHUNKS, DH], BF16, tag="w", name="w_t")
                      src = wdram.ap().rearrange("p (c q) -> p c q", c=ICHUNKS)
                      if split:
                          # chunk 0 lands first so the first proj matmul can start
                          nc.sync.dma_start(wt[:, 0:2, :], src[:, 0:2, :])
                          nc.sync.dma_start(wt[:, 2:ICHUNKS, :], src[:, 2:ICHUNKS, :])
                      else:
                          nc.sync.dma_start(wt[:], src)
                      return wt

                  def dma_x2(xdram, h, split=False):
                      # 2-tile (1024-token) batch: 2 KB DMA lines
                      tsl = slice(1024 * h, 1024 * (h + 1))
                      xt = xtbp.tile([128, ICHUNKS, 1024], BF16, tag="xtb", name="xtb_tile")
                      src = xdram.ap()[:, tsl].rearrange("(c p) q -> p c q", c=ICHUNKS)
                      if split:
                          nc.sync.dma_start(xt[:, 0:2, 0:512], src[:, 0:2, 0:512])
                          nc.sync.dma_start(xt[:, 2:ICHUNKS, 0:512], src[:, 2:ICHUNKS, 0:512])
                          nc.sync.dma_start(xt[:, :, 512:1024], src[:, :, 512:1024])
                      else:
                          nc.sync.dma_start(xt[:], src)
                      return xt

                  def qk_mms(w_t, xtb, t, p):
                      toff = 512 * (t % 2)
                      pg = shps.tile([128, 512], F32, tag="sh", name="pg_t")
                      for c in range(ICHUNKS):
                          nc.tensor.matmul(
                              pg[:], w_t[:, c, 128 * p : 128 * (p + 1)],
                              xtb[:, c, toff : toff + 512],
                              start=(c == 0), stop=(c == ICHUNKS - 1),
                          )
                      return pg

                  def qk_copy(pg, dst, bias_tiles, t, p, on_act=False):
                      if on_act:
                          # PSUM->SBUF + per-partition bias on the ACT engine
                          # (identity shares the exp activation table: no reload)
                          nc.scalar.activation(
                              dst[p][:, 512 * t : 512 * (t + 1)], pg[:],
                              ACTF.Identity, bias=bias_tiles[p], scale=1.0)
                      else:
                          nc.vector.tensor_scalar_add(
                              dst[p][:, 512 * t : 512 * (t + 1)], pg[:], bias_tiles[p])

                  def qk_group(w_t, xtb, dst, bias_tiles, t, p, on_act=False):
                      qk_copy(qk_mms(w_t, xtb, t, p), dst, bias_tiles, t, p, on_act)

                  def v_mms(wv_t, xtb, t, b):
                      toff = 512 * (t % 2)
                      pg = shps.tile([128, 512], F32, tag="sh", name="pg_t")
                      for c in range(ICHUNKS):
                          nc.tensor.matmul(
                              pg[:], xtb[:, c, toff + 128 * b : toff + 128 * (b + 1)],
                              wv_t[:, c, :],
                              start=(c == 0), stop=(c == ICHUNKS - 1),
                          )
                      return pg

                  def v_copy(pg, t, b):
                      nc.vector.scalar_tensor_tensor(
                          V[4 * t + b][:, :, 0:64], pg[:], 1.0, bvb[:],
                          ALU.mult, ALU.add,
                      )

                  def v_group(wv_t, xtb, t, b):
                      v_copy(v_mms(wv_t, xtb, t, b), t, b)

                  if _rep == 0:
                      # pre-heat the PE during the startup DMA window: dummy
                      # matmuls with no DMA deps release the HAM throttle
                      # (1.2 -> 2.4 GHz) before the first real matmul issues
                      hs = wp.tile([1, 512], BF16, tag="heat", name="heat_s")
                      nc.gpsimd.memset(hs[:], 0.0)
                      hp = sps.tile([128, 1024], F32, tag="s01", name="heat_ps")
                      for _h in range(6):
                          nc.tensor.matmul(hp[:, 0:512], hs[0:1, 0:128], hs[:],
                                           start=True, stop=True)

                  # ---- startup: K weights + K x-tiles first, consts behind ----
                  xcache = {}
                  wk_t = load_w(wkt, split=(_rep == 0))
                  xk = dma_x2(xkT, 0, split=(_rep == 0))
                  if _rep == 0:
                      load_consts_early()
                  wq_t = load_w(wqt, split=(_rep == 0))
                  xq = dma_x2(xqT, 0, split=(_rep == 0))
                  if _rep == 0:
                      load_consts_mid()
                  wv_t = load_w(wvt, split=(_rep == 0))
                  xv = dma_x2(xvT, 0, split=(_rep == 0))
                  if _rep == 0:
                      load_consts_late()
                  xcache[0] = {"q": xq, "k": xk, "v": xv}

                  # ---- tile-0 projections emitted directly ----
                  for p in range(PAIRS):
                      qk_group(wk_t, xk, KT, bk_t, 0, p, on_act=True)
                  for p in range(PAIRS):
                      qk_group(wq_t, xq, QT, bq_t, 0, p)
                  # bv broadcast tile [128, 512]
                  pbv = shps.tile([128, 512], F32, tag="sh", name="pbv_tile")
                  nc.tensor.matmul(pbv[:], onescol_t[:], bv_t[:], start=True, stop=True)
                  bvb = wp.tile([128, DH], BF16, tag="bvb", name="bvb_tile")
                  nc.vector.tensor_copy(bvb[:], pbv[:])
                  for b in range(4):
                      v_group(wv_t, xv, 0, b)

                  def stagger(mk_mm, mk_copy, ids):
                      # emit each group's psum->sbuf copy one work-unit after
                      # its matmuls so the in-order ACT/DVE queues never block
                      # a ready exp/mask behind a copy whose psum is pending
                      st = {}
                      units = []
                      for i, idx in enumerate(ids):
                          units.append(lambda idx=idx: st.__setitem__(idx, mk_mm(idx)))
                          if i >= 1:
                              units.append(lambda j=ids[i - 1]: mk_copy(j, st.pop(j)))
                      units.append(lambda j=ids[-1]: mk_copy(j, st.pop(j)))
                      return units

                  def queue_proj(t1):
                      h = t1 // 2
                      kv = []
                      if t1 == 2:
                          def dq(): xcache.setdefault(1, {})["q"] = dma_x2(xqT, 1)
                          projq.append(dq)
                      projq.extend(stagger(
                          lambda p: qk_mms(wq_t, xcache[h]["q"], t1, p),
                          lambda p, pg: qk_copy(pg, QT, bq_t, t1, p),
                          list(range(PAIRS))))
                      if t1 == 2:
                          def dk(): xcache[1]["k"] = dma_x2(xkT, 1)
                          kv.append(dk)
                      kv.extend(stagger(
                          lambda p: qk_mms(wk_t, xcache[h]["k"], t1, p),
                          lambda p, pg: qk_copy(pg, KT, bk_t, t1, p, on_act=True),
                          list(range(PAIRS))))
                      if t1 == 2:
                          def dv(): xcache[1]["v"] = dma_x2(xvT, 1)
                          kv.append(dv)
                      kv.extend(stagger(
                          lambda b: v_mms(wv_t, xcache[h]["v"], t1, b),
                          lambda b, pg: v_copy(pg, t1, b),
                          list(range(4))))
                      if t1 == QT_TILES - 1:
                          # last tile: its own chunks only touch K/V tile t1 from
                          # chunk j=4*t1 (iter 13) on; safe to drain in-stream
                          lateq.extend(kv)
                      else:
                          projq.extend(kv)

                  # ---- attention with interleaved proj/out-proj ----
                  for t in range(QT_TILES):
                      while projq:       # proj for this tile must be emitted
                          projq.pop(0)()
                      while normal and normal[0][0] <= t - 2:
                          # out-proj units two tiles back must emit before this
                          # tile's pair tails reuse their `a` buffers
                          normal.pop(0)[1]()
                      if t < QT_TILES - 1:
                          queue_proj(t + 1)
                      nch = 4 * (t + 1)
                      pace["iter"], pace["total"], pace["next_norm"] = 0, 4 * nch, 0
                      A = []
                      for p in range(PAIRS):
                          psA = accps.tile([128, 512], F32, tag="acc", name="psA_t")
                          psB = accps.tile([128, 512], F32, tag="acc", name="psB_t")
                          for j in range(nch):
                              ksl = slice(128 * j, 128 * (j + 1))
                              m = j - 4 * t
                              q0 = 128 * m if m >= 1 else 0
                              s01 = sps.tile([128, 1024], F32, tag="s01", name="s01_t")
                              nc.tensor.matmul(
                                  s01[:, q0:512], KT[p][0:64, ksl],
                                  QT[p][0:64, 512 * t + q0 : 512 * (t + 1)],
                                  start=True, stop=True,
                              )
                              nc.tensor.matmul(
                                  s01[:, 512 + q0 : 1024], KT[p][64:128, ksl],
                                  QT[p][64:128, 512 * t + q0 : 512 * (t + 1)],
                                  start=True, stop=True,
                              )
                              pb = pbp.tile([128, 1024], BF16, tag="pb", name="pb_t")
                              if q0 == 0:
                                  nc.scalar.activation(pb[:], s01[:], ACTF.Exp, scale=0.125)
                              else:
                                  nc.scalar.activation(
                                      pb[:, q0:512], s01[:, q0:512], ACTF.Exp, scale=0.125)
                                  nc.scalar.activation(
                                      pb[:, 512 + q0 : 1024], s01[:, 512 + q0 : 1024],
                                      ACTF.Exp, scale=0.125)
                              if m >= 0:
                                  msk = mask_ap(m, q0, 512)
                                  nc.vector.tensor_tensor(
                                      pb[:, q0:512], pb[:, q0:512], msk, ALU.mult)
                                  nc.vector.tensor_tensor(
                                      pb[:, 512 + q0 : 1024], pb[:, 512 + q0 : 1024],
                                      msk, ALU.mult)
                              st_ = (j == 0)
                              sp_ = (j == nch - 1)
                              nc.tensor.matmul(
                                  psA[:, q0:512], V[j][:, 2 * p : 2 * p + 1, :],
                                  pb[:, q0:512], start=st_, stop=sp_,
                              )
                              nc.tensor.matmul(
                                  psB[:, q0:512], V[j][:, 2 * p + 1 : 2 * p + 2, :],
                                  pb[:, 512 + q0 : 1024], start=st_, stop=sp_,
                              )
                              pace["iter"] += 1
                              drain_one()

                          def pair_tail(p=p, psA=psA, psB=psB, A=A):
                              r2 = rpool.tile([65, 512], F32R, tag="r", name="r_t")
                              r2b = rpool.tile([65, 512], F32R, tag="r", name="r2b_t")
                              with nc.allow_low_precision(reason="f32r storage is fp32"):
                                  nc.vector.reciprocal(r2[64:65, :], psA[64:65, :])
                                  nc.vector.reciprocal(r2b[64:65, :], psB[64:65, :])
                              rbA = shps.tile([64, 512], F32, tag="sh", name="rbA_t")
                              rbB = shps.tile([64, 512], F32, tag="sh", name="rbB_t")
                              nc.tensor.matmul(rbA[:], selp_t[64:65, 0:64],
                                               r2[64:65, :], start=True, stop=True)
                              nc.tensor.matmul(rbB[:], selp_t[64:65, 0:64],
                                               r2b[64:65, :], start=True, stop=True)
                              rbc = rpool.tile([128, 512], BF16, tag="rbc", name="rbc_t")
                              nc.vector.tensor_copy(rbc[0:64, :], rbA[:])
                              nc.vector.tensor_scalar_mul(rbc[64:128, :], rbB[:], 1.0)
                              a = apool.tile([128, 512], BF16, tag=f"a{p}", name=f"a_t{p}")
                              nc.vector.scalar_tensor_tensor(
                                  a[0:64, :], psA[0:64, :], 1.0, rbc[0:64, :],
                                  ALU.mult, ALU.mult,
                              )
                              nc.vector.scalar_tensor_tensor(
                                  a[64:128, :], psB[0:64, :], 1.0, rbc[64:128, :],
                                  ALU.mult, ALU.mult,
                              )
                              A.append(a)

                          urgent.append(pair_tail)

                      for tl in range(4):
                          for eh in range(2):
                              def oproj(t=t, tl=tl, eh=eh, A=A):
                                  po = shps.tile([128, 512], F32, tag="sh", name="po_t")
                                  for p in range(PAIRS):
                                      nc.tensor.matmul(
                                          po[:],
                                          A[p][:, 128 * tl : 128 * (tl + 1)],
                                          wo_t[p][:, 512 * eh : 512 * (eh + 1)],
                                          start=(p == 0),
                                          stop=(p == PAIRS - 1),
                                      )
                                  ob = osbp.tile([128, 512], BF16, tag="ob", name="ob_t")
                                  nc.vector.tensor_copy(ob[:], po[:])
                                  r0 = 512 * t + 128 * tl
                                  nc.sync.dma_start(
                                      outp.ap()[r0 : r0 + 128, 512 * eh : 512 * (eh + 1)],
                                      ob[:],
                                  )
                              normal.append((t, oproj))

                  for q in (urgent, projq, lateq):
                      while q:
                          q.pop(0)()
                  while normal:
                      normal.pop(0)[1]()

    nc.compile()
    return nc


_RT = {}


def _get_runtime():
    if "rt" in _RT:
        return _RT["rt"]

    import jax
    import numpy as np
    from jax.experimental.shard_map import shard_map
    from jax.sharding import Mesh, PartitionSpec

    import concourse.mybir as mybir
    from concourse.bass2jax import (
        _bass_exec_p,
        install_neuronx_cc_hook,
        partition_id_tensor,
    )

    nc = _build_nc()
    install_neuronx_cc_hook()

    partition_name = nc.partition_id_tensor.name if nc.partition_id_tensor else None
    in_names, out_names, out_avals, zero_shapes = [], [], [], []
    for alloc in nc.m.functions[0].allocations:
        if not isinstance(alloc, mybir.MemoryLocationSet):
            continue
        if not alloc.memorylocations:
            continue
        name = alloc.memorylocations[0].name
        if alloc.kind == "ExternalInput":
            if name != partition_name:
                in_names.append(name)
        elif alloc.kind == "ExternalOutput":
            shape = tuple(alloc.tensor_shape)
            dtype = mybir.dt.np(alloc.dtype)
            out_names.append(name)
            out_avals.append(jax.core.ShapedArray(shape, dtype))
            zero_shapes.append((shape, dtype))
    n_params = len(in_names)
    n_outs = len(out_names)
    all_in_names = list(in_names) + list(out_names)
    if partition_name is not None:
        all_in_names.append(partition_name)
    donate = tuple(range(n_params, n_params + n_outs))

    def _body(*args):
        operands = list(args)
        if partition_name is not None:
            operands.append(partition_id_tensor())
        outs = _bass_exec_p.bind(
            *operands,
            out_avals=tuple(out_avals),
            in_names=tuple(all_in_names),
            out_names=tuple(out_names),
            lowering_input_output_aliases=(),
            sim_require_finite=True,
            sim_require_nnan=True,
            nc=nc,
        )
        return tuple(outs)

    devices = jax.devices()[:NCORES]
    assert len(devices) == NCORES
    mesh = Mesh(np.asarray(devices), ("core",))
    in_specs = (PartitionSpec("core"),) * (n_params + n_outs)
    out_specs = (PartitionSpec("core"),) * n_outs
    fn = jax.jit(
        shard_map(_body, mesh=mesh, in_specs=in_specs, out_specs=out_specs,
                  check_rep=False),
        donate_argnums=donate,
        keep_unused=True,
    )
    rt = {
        "fn": fn,
        "in_names": in_names,
        "out_names": out_names,
        "zero_shapes": zero_shapes,
        "n_params": n_params,
        "mesh": mesh,
        "nc": nc,
    }
    _RT["rt"] = rt
    return rt


def _make_masks():
    # causal base: maskb[r, u] = 1[u >= r + 384]; mask_m = maskb[:, 384-128m:...]
    r = np.arange(128, dtype=np.int64)[:, None]
    u = np.arange(896, dtype=np.int64)[None, :]
    return (u >= r + 384).astype(np.float32)


def _shard_inputs(query, key, value, Wq, bq, Wk, bk, Wv, bv, Wo, bo, pad_mask):
    f = np.float32
    import ml_dtypes
    bf = ml_dtypes.bfloat16
    query = np.asarray(query, f).reshape(B, S, D)
    key = np.asarray(key, f).reshape(B, S, D)
    value = np.asarray(value, f).reshape(B, S, D)
    sel2 = np.zeros((66, 128), f)
    sel2[64, :] = 1.0
    consts = {
        "onescol": np.ones((1, 128), bf),
        "selp": sel2,
        "masks": _make_masks().astype(bf),
    }
    xT = {b: {
        "xqT": query[b].T.astype(bf),
        "xkT": key[b].T.astype(bf),
        "xvT": value[b].T.astype(bf),
    } for b in range(B)}
    wT = {
        "q": np.asarray(Wq, f).T,
        "k": np.asarray(Wk, f).T,
        "v": np.asarray(Wv, f).T,
        "o": np.asarray(Wo, f).T,
    }

    def w_pre(w):
        # [1024, 512] -> [128, 8*512]: row p, block c = w[128c+p, :]
        return np.ascontiguousarray(
            w.reshape(ICHUNKS, 128, DH).transpose(1, 0, 2).reshape(128, ICHUNKS * DH)
        )

    in_maps = []
    for c in range(NCORES):
        b = c // 2
        hh = c % 2
        sl = slice(DH * hh, DH * (hh + 1))
        m = {
            **xT[b],
            "wqt": w_pre(wT["q"][:, sl]).astype(bf),
            "wkt": w_pre(wT["k"][:, sl]).astype(bf),
            "wvt": w_pre(wT["v"][:, sl]).astype(bf),
            "wot": wT["o"][sl, :].astype(bf),
            "bqk": np.ascontiguousarray(np.concatenate(
                [np.asarray(bq, f)[sl].reshape(4, 128).T,
                 np.asarray(bk, f)[sl].reshape(4, 128).T], axis=1)),
            "bv": np.asarray(bv, f)[sl].reshape(1, DH).astype(bf),
            **consts,
        }
        in_maps.append(m)
    return in_maps


def _run(rt, in_maps):
    import jax
    import numpy as np

    n = rt["n_params"]
    concat_in = [
        np.concatenate([np.asarray(in_maps[c][name]) for c in range(NCORES)], axis=0)
        for name in rt["in_names"]
    ]
    concat_zeros = [
        np.zeros((NCORES * sh[0], *sh[1:]), dt) for sh, dt in rt["zero_shapes"]
    ]
    out_arrs = rt["fn"](*concat_in, *concat_zeros)
    res = []
    for c in range(NCORES):
        d = {}
        for i, name in enumerate(rt["out_names"]):
            sh = rt["zero_shapes"][i][0]
            d[name] = np.asarray(out_arrs[i]).reshape(NCORES, *sh)[c]
        res.append(d)
    return res


def kernel(**inputs):
    rt = _get_runtime()
    in_maps = _shard_inputs(**inputs)
    res = _run(rt, in_maps)
    bo = np.asarray(inputs["bo"], np.float32)
    out = np.empty((B, S, D), dtype=np.float32)
    for b in range(B):
        out[b] = (
            np.asarray(res[2 * b]["outp"], np.float32)
            + np.asarray(res[2 * b + 1]["outp"], np.float32)
            + bo
        )
    return out

